# revision 11
# baseline (speedup 1.0000x reference)
"""Causal single-head attention (B=4, S=2048, D=1024) on 8 Trainium2 NeuronCores.

Sharding: core = (batch, parity). Each batch's 8 query-chunks of 256 are split
{0,3,4,7} / {1,2,5,6} across its two cores so causal work balances exactly.

Algebraic restructure vs the naive QKV form (saves 40% of PE work):
  scores = (x Wq^T)(x Wk^T)^T = x (Wq^T Wk) x^T = x M x^T
with M = Wq^T Wk precomputed on the HOST — the K projection disappears and
scores contract q' = x_q M directly against raw x^T. Likewise
  out = P v = P (x Wv^T) = (P x) Wv^T
so the V projection disappears and P contracts against raw x; the small
(Px) Wv^T projection runs once per query block. Per-core PE work drops from
8.05G to 4.83G MACs with no cross-core communication.

The scores matmul runs in fp8-e4m3 DoubleRow mode (2x PE throughput;
~1.5% logit noise which the 2e-2 rel-err budget absorbs — adding fp8 to the
q' projection too was measured at 2.1e-2, over budget, so that stays bf16).
The post-softmax chain (Px, (Px)Wv) stays bf16 since p/v-side quantization
error propagates 1:1 into the output. The softmax denominator uses fp8
DoubleRow (its error averages down over keys). Scale folding: m = G*32 on
host (so q' lands at std ~10 in e4m3's sweet spot when quantized), exp
applies 1/1024 (= softmax 1/32 x 1/32 descale).

Device algorithm per core (fp32 PSUM accumulation):
  q'T[j,q]   = m_t.T @ xTq           bf16     (1.07G)  per L-chunk, -> fp8
  s_T[sk,sq] = xT8_blk.T @ q'T8      fp8-DR   (1.34G)
  p = exp(s_T/1024) * mask           (no max-subtraction: logits are O(1))
  den[sq]    = p8_blk.T @ ones       fp8-DR
  PxT[i,sq]  = xS_blk.T @ p_blk      bf16     (1.34G)
  out[sq,o]  = (PxT.T @ WvT) / den   bf16     (1.07G)
"""

import sys

if "/opt/trn_rl_repo" not in sys.path:
    sys.path.insert(0, "/opt/trn_rl_repo")

import numpy as np
import ml_dtypes

import concourse.mybir as mybir
import concourse.tile as tile
from concourse import bacc
from concourse.bass_utils import run_bass_kernel_spmd

bf16 = ml_dtypes.bfloat16
f8 = ml_dtypes.float8_e4m3

B, S, D = 4, 2048, 1024
CH = 512            # xT column-chunk width (SBUF tile granularity)
QC = 256            # query-chunk width
BLK = 128           # key-block
# Per-core schedule: 4 query-chunks of 256, processed with a fixed padded
# k-block count (4,8,12,16). Host assigns real chunks sorted by causal depth
# so padding waste is exactly 4 blocks/core; masks (data) encode reality.
SCHED = (4, 8, 12, 16)
MASK_BASE = (0, 4, 12, 24)
NMASK = sum(SCHED)  # 40
DT8 = D // 128      # contraction tiles (bf16 path)
DP4 = D // 256      # DoubleRow contraction pairs (fp8 path)
N_CORES = 8
DT_BF = mybir.dt.bfloat16
DT_F8 = mybir.dt.float8e4
DT_F32 = mybir.dt.float32
DR = mybir.MatmulPerfMode.DoubleRow

_NC_CACHE = {}


def _emit(tc, xT8, xTq, xS, m, wvT, msk, out):
    nc = tc.nc
    Exp = mybir.ActivationFunctionType.Exp

    with (
        tc.tile_pool(name="const", bufs=1) as constp,
        tc.tile_pool(name="sb", bufs=1) as sb,
        tc.tile_pool(name="outs_sb", bufs=2) as osb,
        tc.tile_pool(name="sps", bufs=3, space="PSUM") as sps,
        tc.tile_pool(name="pxps", bufs=2, space="PSUM") as pxps,
        tc.tile_pool(name="ops", bufs=1, space="PSUM") as ops,
        tc.tile_pool(name="dps", bufs=1, space="PSUM") as dps,
    ):
        ones8 = constp.tile([128, 2, 1], DT_F8, tag="ones8", name="ones8")
        nc.vector.memset(ones8, 1.0)

        # fp8 DoubleRow operand tiles: [128, 2, cols]; dim1 is the pair of
        # 128-deep contraction subtiles (d rows 256*pair+128*t+p).
        xT8_t = [
            [sb.tile([128, 2, CH], DT_F8, tag=f"xT{p}_{sc}", name=f"xT{p}_{sc}")
             for sc in range(S // CH)]
            for p in range(DP4)
        ]
        m_t = [
            [sb.tile([128, 128], DT_BF, tag=f"m{i}_{ot}", name=f"m{i}_{ot}")
             for ot in range(DT8)]
            for i in range(DT8)
        ]
        xTq_t = [
            [sb.tile([128, QC], DT_BF, tag=f"xq{i}_{L}", name=f"xq{i}_{L}")
             for L in range(4)]
            for i in range(DT8)
        ]
        q8_t = [sb.tile([128, 2, D], DT_F8, tag=f"q8{p}", name=f"q8{p}")
                for p in range(DP4)]
        xS_t = [sb.tile([128, D], DT_BF, tag=f"xS{b}", name=f"xS{b}")
                for b in range(S // 128)]
        wv_t = [sb.tile([128, D], DT_BF, tag=f"wv{i}", name=f"wv{i}")
                for i in range(DT8)]
        msk_t = [sb.tile([128, QC], DT_BF, tag=f"msk{n}", name=f"msk{n}")
                 for n in range(NMASK)]
        # p tiles pair two L-chunks side by side (cols 0:256 = even L,
        # 256:512 = odd L) so one Px matmul covers both while the shared
        # k-blocks last; beyond the even L's schedule only cols 256:512 live.
        p01_t = [sb.tile([128, 2 * QC], DT_BF, tag=f"p01_{b}", name=f"p01_{b}")
                 for b in range(SCHED[1])]
        p23_t = [sb.tile([128, 2 * QC], DT_BF, tag=f"p23_{b}", name=f"p23_{b}")
                 for b in range(SCHED[3])]
        # fp8 copy of p for the DoubleRow denominator matmul
        p8_t = [sb.tile([128, SCHED[L], QC], DT_F8, tag=f"p8_{L}", name=f"p8_{L}")
                for L in range(4)]
        px_t = {}
        for L in range(4):
            for i in range(DT8):
                px_t[(L, i)] = sb.tile([128, QC], DT_BF, tag=f"px{L}_{i}",
                                       name=f"px{L}_{i}")

        def p_slice(L, b):
            t = p01_t if L < 2 else p23_t
            return t[b][:, QC * (L % 2) : QC * (L % 2 + 1)]

        # DMAs in consumption order so the first matmuls start early; spread
        # across four engines' DMA queues for bandwidth.
        _dmaq = [nc.sync, nc.gpsimd, nc.scalar]

        def _dma(n, dst, src):
            _dmaq[n % 3].dma_start(out=dst, in_=src)

        def dma_m(ot):
            for i in range(DT8):
                _dma(i + ot, m_t[i][ot],
                     m[128 * i : 128 * (i + 1), 128 * ot : 128 * (ot + 1)])

        def dma_xTq(L):
            for i in range(DT8):
                _dma(i + L + 1, xTq_t[i][L],
                     xTq[128 * i : 128 * (i + 1), QC * L : QC * (L + 1)])

        def dma_xT8(sc):
            for p in range(DP4):
                for t in range(2):
                    _dma(2 * p + t + sc, xT8_t[p][sc][:, t, :],
                         xT8[256 * p + 128 * t : 256 * p + 128 * t + 128,
                             CH * sc : CH * (sc + 1)])

        def dma_xS(b0, b1):
            for b in range(b0, b1):
                _dma(b, xS_t[b], xS[128 * b : 128 * (b + 1), :])

        def dma_msk(L):
            for b in range(SCHED[L]):
                _dma(b, msk_t[MASK_BASE[L] + b], msk[MASK_BASE[L] + b])

        def dma_wv():
            for i in range(DT8):
                _dma(i + 1, wv_t[i], wvT[128 * i : 128 * (i + 1), :])

        dma_m(0)
        dma_xTq(0)
        dma_xT8(0)
        for ot in range(1, DT8):
            dma_m(ot)
        dma_msk(0)
        dma_xS(0, 4)
        dma_xTq(1)
        dma_xT8(1)
        dma_msk(1)
        dma_xS(4, 8)
        dma_wv()
        dma_xTq(2)
        dma_xT8(2)
        dma_msk(2)
        dma_xS(8, 12)
        dma_xTq(3)
        dma_xT8(3)
        dma_msk(3)
        dma_xS(12, 16)

        # ---- PE pipeline ----
        def q_proj(L):
            # q'T[j, q(L)] = sum_i m[i].T @ xTq[i, chunk L]  (bf16 -> fp8)
            for ot in range(DT8):
                ps = sps.tile([128, QC], DT_F32, tag="sps", name="sps")
                for i in range(DT8):
                    nc.tensor.matmul(
                        ps,
                        lhsT=m_t[i][ot],
                        rhs=xTq_t[i][L],
                        start=(i == 0),
                        stop=(i == DT8 - 1),
                    )
                nc.scalar.copy(
                    out=q8_t[ot // 2][:, ot % 2, QC * L : QC * (L + 1)], in_=ps
                )

        def scores(L):
            for b in range(SCHED[L]):
                ps = sps.tile([128, QC], DT_F32, tag="sps", name="sps")
                for p in range(DP4):
                    nc.tensor.matmul(
                        ps,
                        lhsT=xT8_t[p][b // 4][:, :, BLK * (b % 4) : BLK * (b % 4 + 1)],
                        rhs=q8_t[p][:, :, QC * L : QC * (L + 1)],
                        perf_mode=DR,
                        start=(p == 0),
                        stop=(p == DP4 - 1),
                    )
                es = osb.tile([128, QC], DT_BF, tag="es", name="es")
                nc.scalar.activation(es, ps, Exp, scale=1.0 / 1024.0)
                mk = msk_t[MASK_BASE[L] + b]
                nc.vector.tensor_mul(p_slice(L, b), es, mk)
                nc.gpsimd.tensor_mul(p8_t[L][:, b, :], es, mk)

        def px(Le):
            # PxT[i, q] for the L-pair (Le, Le+1): one matmul covers both
            # chunks' q-columns while b < SCHED[Le]; beyond that only the
            # odd chunk's half accumulates.
            nb_e, nb_o = SCHED[Le], SCHED[Le + 1]
            pt = p01_t if Le == 0 else p23_t
            for i in range(DT8):
                ps = pxps.tile([128, 2 * QC], DT_F32, tag="pxps", name="pxps")
                for b in range(nb_o):
                    w = xS_t[b][:, 128 * i : 128 * (i + 1)]
                    if b < nb_e:
                        nc.tensor.matmul(
                            ps, lhsT=w, rhs=pt[b],
                            start=(b == 0), stop=(b == nb_o - 1),
                            skip_group_check=True,
                        )
                    else:
                        nc.tensor.matmul(
                            ps[:, QC : 2 * QC], lhsT=w, rhs=pt[b][:, QC : 2 * QC],
                            start=False, stop=(b == nb_o - 1),
                            skip_group_check=True,
                        )
                if i % 2 == 0:
                    nc.scalar.copy(out=px_t[(Le, i)], in_=ps[:, 0:QC])
                    nc.vector.tensor_copy(out=px_t[(Le + 1, i)], in_=ps[:, QC : 2 * QC])
                else:
                    nc.vector.tensor_copy(out=px_t[(Le, i)], in_=ps[:, 0:QC])
                    nc.scalar.copy(out=px_t[(Le + 1, i)], in_=ps[:, QC : 2 * QC])

        def den_out(L):
            for sqt in range(QC // 128):
                pd = dps.tile([128, 1], DT_F32, tag="pd", name="pd")
                for u in range(SCHED[L] // 2):
                    nc.tensor.matmul(
                        pd,
                        lhsT=p8_t[L][:, 2 * u : 2 * u + 2,
                                     128 * sqt : 128 * (sqt + 1)],
                        rhs=ones8,
                        perf_mode=DR,
                        start=(u == 0),
                        stop=(u == SCHED[L] // 2 - 1),
                    )
                po = ops.tile([128, D], DT_F32, tag="po", name="po")
                for i in range(DT8):
                    pxs = px_t[(L, i)][:, 128 * sqt : 128 * (sqt + 1)]
                    nc.tensor.matmul(
                        po[:, 0:CH], lhsT=pxs, rhs=wv_t[i][:, 0:CH],
                        start=(i == 0), stop=(i == DT8 - 1),
                        skip_group_check=True,
                    )
                    nc.tensor.matmul(
                        po[:, CH:D], lhsT=pxs, rhs=wv_t[i][:, CH:D],
                        start=(i == 0), stop=(i == DT8 - 1),
                        skip_group_check=True,
                    )
                r = osb.tile([128, 1], DT_F32, tag="r", name="r")
                nc.vector.reciprocal(r, pd)
                o = osb.tile([128, D], DT_BF, tag="osb", name="osb")
                nc.vector.tensor_scalar_mul(o, po, r)
                nc.sync.dma_start(
                    out=out[QC * L + 128 * sqt : QC * L + 128 * (sqt + 1), :],
                    in_=o,
                )

        q_proj(0)
        scores(0)
        q_proj(1)
        scores(1)
        px(0)
        den_out(0)
        den_out(1)
        q_proj(2)
        scores(2)
        q_proj(3)
        scores(3)
        px(2)
        den_out(2)
        den_out(3)


def build_program():
    nc = bacc.Bacc(
        "TRN2",
        target_bir_lowering=False,
        debug=False,
        enable_asserts=False,
        num_devices=N_CORES,
    )
    xT8 = nc.dram_tensor("xT8", [D, S], DT_F8, kind="ExternalInput").ap()
    xTq = nc.dram_tensor("xTq", [D, 4 * QC], DT_BF, kind="ExternalInput").ap()
    xS = nc.dram_tensor("xS", [S, D], DT_BF, kind="ExternalInput").ap()
    m = nc.dram_tensor("m", [D, D], DT_BF, kind="ExternalInput").ap()
    wvT = nc.dram_tensor("wvT", [D, D], DT_BF, kind="ExternalInput").ap()
    msk = nc.dram_tensor("msk", [NMASK, BLK, QC], DT_BF, kind="ExternalInput").ap()
    out = nc.dram_tensor("out", [4 * QC, D], DT_BF, kind="ExternalOutput").ap()
    with tile.TileContext(nc) as tc:
        _emit(tc, xT8, xTq, xS, m, wvT, msk, out)
    nc.compile()
    return nc


def get_program():
    if "nc" not in _NC_CACHE:
        _NC_CACHE["nc"] = build_program()
    return _NC_CACHE["nc"]


def _chunks_for(core):
    """Per-core 256-wide query chunks, L-ordered to match SCHED=(4,8,12,16).
    Real causal k-block need: chunk j -> 2(j+1)."""
    return [0, 3, 4, 7] if core % 2 == 0 else [1, 2, 5, 6]


def _build_masks(chunks, permuted):
    """[40,128,256] in {0,1}: allowed iff actual_key <= actual_query, where
    for odd cores the key axis is permuted by pos^256 (see build_in_maps).
    Padding blocks beyond a chunk's real causal depth come out all-zero."""
    m = np.zeros((NMASK, BLK, QC), np.float32)
    p = np.arange(BLK)[:, None]
    c = np.arange(QC)[None, :]
    for L, j in enumerate(chunks):
        for b in range(SCHED[L]):
            sk = BLK * b + p
            if permuted:
                sk = sk ^ 256
            m[MASK_BASE[L] + b] = sk <= QC * j + c
    return m.astype(bf16)


def _perm256(a, axis):
    """Swap the 256-halves of every 512-chunk along `axis` (pos -> pos^256)."""
    sh = a.shape
    n = sh[axis]
    new_shape = sh[:axis] + (n // 512, 2, 256) + sh[axis + 1 :]
    return np.ascontiguousarray(
        np.flip(a.reshape(new_shape), axis=axis + 1).reshape(sh)
    )


def build_in_maps(x, Wq, Wk, Wv):
    Wq = np.asarray(Wq, np.float32)
    Wk = np.asarray(Wk, np.float32)
    Wv = np.asarray(Wv, np.float32)
    # M = Wq^T Wk; scale x32 here (so q' quantizes to fp8 at std ~10) and
    # fold softmax 1/32 plus the 1/32 descale into the device exp's 1/1024.
    m = ((Wq.T @ Wk) * 32.0).astype(bf16)
    wv = np.ascontiguousarray(Wv.T).astype(bf16)
    masks = {par: _build_masks(_chunks_for(par), par == 1) for par in (0, 1)}
    in_maps = []
    for core in range(N_CORES):
        b = core // 2
        xb = np.asarray(x[b], np.float32).astype(bf16)  # [S, D]
        xT = np.ascontiguousarray(np.asarray(x[b], np.float32).T)
        xq = np.concatenate(
            [xT[:, QC * j : QC * (j + 1)] for j in _chunks_for(core)], axis=1
        ).astype(bf16)
        if core % 2 == 1:
            # Key/seq-permute by pos^256 so both parities share one
            # instruction stream; xT8 columns, xS rows and mask key
            # coordinates move together (q-side xTq is gathered on host and
            # needs no permutation).
            xT = _perm256(xT, 1)
            xb = _perm256(xb, 0)
        in_maps.append(
            {"xT8": xT.astype(f8), "xTq": xq, "xS": xb, "m": m, "wvT": wv,
             "msk": masks[core % 2]}
        )
    return in_maps


def assemble_output(results):
    out = np.zeros((B, S, D), np.float32)
    for core in range(N_CORES):
        b = core // 2
        for L, j in enumerate(_chunks_for(core)):
            out[b, QC * j : QC * (j + 1)] = \
                results[core]["out"][QC * L : QC * (L + 1)].astype(np.float32)
    return out


def kernel(x, Wq, Wk, Wv):
    x = np.asarray(x, np.float32)
    nc = get_program()
    in_maps = build_in_maps(x, np.asarray(Wq, np.float32),
                            np.asarray(Wk, np.float32), np.asarray(Wv, np.float32))
    res = run_bass_kernel_spmd(nc, in_maps, core_ids=list(range(N_CORES)))
    return assemble_output(res.results)
